# revision 13
# baseline (speedup 1.0000x reference)
"""Trainium2 Bass kernel for nn_GCN_32289564131895 (gnn_message_passing).

8 NeuronCores, node-sharded (512 rows/core), weights replicated, on-device
collectives. Key ideas:

- Dense masked adjacency: top-32 selection becomes a per-row threshold tau
  (exact: per-block top-32 candidates always cover the row top-32; the merge
  of the per-chunk candidates yields the exact 32nd max). W = adj*(adj>=tau).
  The dinv[idx] gather becomes a column scale folded into the gathered P/Q
  operands; both gather-einsums become dense matmuls against W / W.T.
- mm1/mm2 exact via fp16 hi/lo 3-pass (top-k boundary gaps go down to 2e-8;
  any selection flip costs ~1e-2 final rel err, so selection must be exact).
- adj kept fully in SBUF (no DRAM spill); b2 bias added as a partition-
  broadcast row on DVE (no PE bias matmuls).
- mm2 j-chunks sized [1024x3, 512, 256, 256] so the trailing top-k DVE
  backlog after the last matmul is short; phase-0 (P=node@cw1) runs there.
- dinv_own folded into P rows before a single combined P+deg AllGather;
  tail uses stats AllReduce + out1 AllGather (BN applied post-gather in
  [c, j] layout), every core redundantly computes Q = bn@cw2 - no Q gather.
"""

import sys

for _p in ("/opt/trn_rl_repo", "/root/.axon_site/_ro/trn_rl_repo"):
    if _p not in sys.path:
        sys.path.insert(0, _p)

from contextlib import ExitStack

import numpy as np

import concourse.bass as bass
import concourse.mybir as mybir
import concourse.tile as tile
from concourse import bacc
from concourse.bass_utils import run_bass_kernel_spmd
from concourse.masks import make_identity

dt = mybir.dt
AF = mybir.ActivationFunctionType
ALU = mybir.AluOpType

N_CORES = 8
N = 4096
D = 4096
H_MLP = 1024
HID = 256
OUT = 256
BN_EPS = 1e-5

KIN = D + 4 + 1001        # 5101
KIN_PAD = 5120
ROWS = N // N_CORES       # 512
RT = ROWS // 128          # 4
KT1 = KIN_PAD // 128      # 40
NT = H_MLP // 128         # 8
JT = N // 128              # 32
CT = HID // 128            # 2
FT = D // 128              # 32

# mm2 j-chunks: big first, small last (shrinks trailing DVE top-k backlog)
CHUNKS = [(0, 1024), (1024, 1024), (2048, 1024), (3072, 512), (3584, 256),
          (3840, 256)]
NCH = len(CHUNKS)
PC = HID + 2              # P row + fp16 deg + pad, for the combined AllGather

TRACE = False
LAST_INFO = {}
_CACHED_NC = None


def _build():
    nc = bacc.Bacc(None, target_bir_lowering=False)
    f32 = dt.float32
    fp16 = dt.float16

    at_h = nc.declare_dram_parameter("at_h", [KIN_PAD, ROWS], fp16, isOutput=False)
    at_l = nc.declare_dram_parameter("at_l", [KIN_PAD, ROWS], fp16, isOutput=False)
    w1h = nc.declare_dram_parameter("w1h", [KIN_PAD, H_MLP], fp16, isOutput=False)
    w1l = nc.declare_dram_parameter("w1l", [KIN_PAD, H_MLP], fp16, isOutput=False)
    b1 = nc.declare_dram_parameter("b1", [H_MLP], f32, isOutput=False)
    w2h = nc.declare_dram_parameter("w2h", [H_MLP, N], fp16, isOutput=False)
    w2l = nc.declare_dram_parameter("w2l", [H_MLP, N], fp16, isOutput=False)
    b2 = nc.declare_dram_parameter("b2", [N], f32, isOutput=False)
    nodet = nc.declare_dram_parameter("nodet", [D, ROWS], fp16, isOutput=False)
    cw1 = nc.declare_dram_parameter("cw1", [D, HID], fp16, isOutput=False)
    b1c = nc.declare_dram_parameter("b1c", [HID], f32, isOutput=False)
    cw2 = nc.declare_dram_parameter("cw2", [HID, OUT], fp16, isOutput=False)
    b2c = nc.declare_dram_parameter("b2c", [OUT], f32, isOutput=False)
    gamma = nc.declare_dram_parameter("gamma", [HID], f32, isOutput=False)
    beta = nc.declare_dram_parameter("beta", [HID], f32, isOutput=False)
    out = nc.declare_dram_parameter("out", [OUT, ROWS], f32, isOutput=True)

    # internal DRAM
    pdeg_shard = nc.dram_tensor("pdeg_shard", [ROWS, PC], fp16)
    pdeg_full = nc.dram_tensor("pdeg_full", [N, PC], fp16, addr_space="Shared")
    stats_loc = nc.dram_tensor("stats_loc", [4 * 128], f32)
    stats_red = nc.dram_tensor("stats_red", [4 * 128], f32, addr_space="Shared")
    o1_shard = nc.dram_tensor("o1_shard", [HID, ROWS], fp16)
    o1_full = nc.dram_tensor("o1_full", [N_CORES * HID, ROWS], fp16,
                             addr_space="Shared")

    GRP = [list(range(N_CORES))]

    with tile.TileContext(nc) as tc:
        with (
            tc.tile_pool(name="const", bufs=1) as const,
            tc.tile_pool(name="hold", bufs=1) as hold,
            tc.tile_pool(name="wstage", bufs=4) as wstage,
        ):
            # ---------------- constants ----------------
            b1_sb = const.tile([128, NT], f32, tag="b1")
            nc.sync.dma_start(b1_sb[:], b1.rearrange("(t p) -> p t", p=128))
            b2b = const.tile([128, N], f32, tag="b2b")
            nc.sync.dma_start(
                b2b[:],
                b2.rearrange("(o j) -> o j", o=1).broadcast_to([128, N]))
            b1c_sb = const.tile([128, CT], f32, tag="b1c")
            nc.sync.dma_start(b1c_sb[:], b1c.rearrange("(t p) -> p t", p=128))
            b2c_sb = const.tile([128, CT], f32, tag="b2c")
            nc.sync.dma_start(b2c_sb[:], b2c.rearrange("(t p) -> p t", p=128))
            gam_sb = const.tile([128, CT], f32, tag="gam")
            nc.sync.dma_start(gam_sb[:], gamma.rearrange("(t p) -> p t", p=128))
            bet_sb = const.tile([128, CT], f32, tag="bet")
            nc.sync.dma_start(bet_sb[:], beta.rearrange("(t p) -> p t", p=128))
            cw2_sb = const.tile([128, CT, OUT], fp16, tag="cw2")
            nc.sync.dma_start(cw2_sb[:], cw2.rearrange("(t p) c -> p t c", p=128))
            ident = const.tile([128, 128], f32, tag="ident")
            make_identity(nc, ident[:])
            # per-(i,chunk)-block top-32 candidate values
            btop = const.tile([128, RT, NCH, 32], f32, tag="btop")

            sstack = ExitStack()
            st3 = sstack.enter_context(tc.tile_pool(name="st3", bufs=4))
            st2 = sstack.enter_context(tc.tile_pool(name="st2", bufs=3))
            stage = sstack.enter_context(tc.tile_pool(name="stage", bufs=2))

            adjp = ExitStack()
            adjpool = adjp.enter_context(tc.tile_pool(name="adjpool", bufs=1))
            adjs = adjpool.tile([128, RT, N], f32, tag="adjs")

            # ---------------- phase 1: mm1 (exact): h.T[n, i] ----------------
            ps1 = ExitStack()
            ph = ps1.enter_context(tc.tile_pool(name="ph", bufs=8, space="PSUM"))
            psum_h = [ph.tile([128, ROWS], f32, tag="ph", name=f"psum_h{_i}")
                      for _i in range(NT)]
            for k in range(KT1):
                ath_t = st3.tile([128, ROWS], fp16, tag="ath")
                nc.sync.dma_start(ath_t[:], at_h[k * 128:(k + 1) * 128, :])
                atl_t = st3.tile([128, ROWS], fp16, tag="atl")
                nc.sync.dma_start(atl_t[:], at_l[k * 128:(k + 1) * 128, :])
                w1h_t = st2.tile([128, H_MLP], fp16, tag="wA")
                nc.sync.dma_start(w1h_t[:], w1h[k * 128:(k + 1) * 128, :])
                w1l_t = st2.tile([128, H_MLP], fp16, tag="wB")
                nc.sync.dma_start(w1l_t[:], w1l[k * 128:(k + 1) * 128, :])
                for n in range(NT):
                    nc.tensor.matmul(
                        psum_h[n][:],
                        lhsT=w1h_t[:, n * 128:(n + 1) * 128],
                        rhs=ath_t[:],
                        start=(k == 0), stop=False,
                    )
                    nc.tensor.matmul(
                        psum_h[n][:],
                        lhsT=w1h_t[:, n * 128:(n + 1) * 128],
                        rhs=atl_t[:],
                        start=False, stop=False,
                    )
                    nc.tensor.matmul(
                        psum_h[n][:],
                        lhsT=w1l_t[:, n * 128:(n + 1) * 128],
                        rhs=ath_t[:],
                        start=False, stop=(k == KT1 - 1),
                    )
            hh_sb = hold.tile([128, NT, ROWS], fp16, tag="hh")
            hl_sb = hold.tile([128, NT, ROWS], fp16, tag="hl")
            for n in range(NT):
                hup = stage.tile([128, ROWS], f32, tag="hup")
                nc.scalar.activation(hup[:], psum_h[n][:], AF.Relu,
                                     bias=b1_sb[:, n:n + 1], scale=1.0)
                nc.vector.tensor_copy(hh_sb[:, n, :], hup[:])
                hup2 = stage.tile([128, ROWS], f32, tag="hup2")
                nc.vector.tensor_copy(hup2[:], hh_sb[:, n, :])
                nc.vector.tensor_sub(hup2[:], hup[:], hup2[:])
                nc.vector.tensor_copy(hl_sb[:, n, :], hup2[:])
            ps1.close()

            # ------- phase 2: mm2 (exact) + bias + block top-32 candidates ---
            ps2 = ExitStack()
            pa = ps2.enter_context(tc.tile_pool(name="pa", bufs=4, space="PSUM"))
            for ci, (off, w) in enumerate(CHUNKS):
                psum_a = [pa.tile([128, w], f32, tag="pa",
                                  name=f"psum_a{ci}_{_i}") for _i in range(RT)]
                for n in range(NT):
                    w2h_t = st2.tile([128, 1024], fp16, tag="wA")
                    nc.sync.dma_start(
                        w2h_t[:, 0:w], w2h[n * 128:(n + 1) * 128, off:off + w])
                    w2l_t = st2.tile([128, 1024], fp16, tag="wB")
                    nc.sync.dma_start(
                        w2l_t[:, 0:w], w2l[n * 128:(n + 1) * 128, off:off + w])
                    for i in range(RT):
                        for p0 in range(0, w, 512):
                            pw = min(512, w - p0)
                            nc.tensor.matmul(
                                psum_a[i][:, p0:p0 + pw],
                                lhsT=hh_sb[:, n, i * 128:(i + 1) * 128],
                                rhs=w2h_t[:, p0:p0 + pw],
                                start=(n == 0), stop=False,
                            )
                            nc.tensor.matmul(
                                psum_a[i][:, p0:p0 + pw],
                                lhsT=hh_sb[:, n, i * 128:(i + 1) * 128],
                                rhs=w2l_t[:, p0:p0 + pw],
                                start=False, stop=False,
                            )
                            nc.tensor.matmul(
                                psum_a[i][:, p0:p0 + pw],
                                lhsT=hl_sb[:, n, i * 128:(i + 1) * 128],
                                rhs=w2h_t[:, p0:p0 + pw],
                                start=False, stop=(n == NT - 1),
                            )
                for i in range(RT):
                    asl = adjs[:, i, off:off + w]
                    # psum + b2 (exact f32 add, b2 broadcast across partitions)
                    nc.vector.tensor_add(asl, psum_a[i][:], b2b[:, off:off + w])
                    # block top-32 candidates (max8/match_replace x4)
                    zb = stage.tile([128, 1024], f32, tag="scr")
                    m8 = wstage.tile([128, 8], f32, tag="m8")
                    nc.vector.max(out=m8[:], in_=asl)
                    nc.vector.tensor_copy(btop[:, i, ci, 0:8], m8[:])
                    nc.vector.match_replace(out=zb[:, 0:w], in_to_replace=m8[:],
                                            in_values=asl, imm_value=-1e30)
                    for r in range(1, 4):
                        nc.vector.max(out=m8[:], in_=zb[:, 0:w])
                        nc.vector.tensor_copy(btop[:, i, ci, 8 * r:8 * r + 8],
                                              m8[:])
                        if r < 3:
                            nc.vector.match_replace(
                                out=zb[:, 0:w], in_to_replace=m8[:],
                                in_values=zb[:, 0:w], imm_value=-1e30)
            ps2.close()

            # ------- phase 0 (moved late): P = node_emb @ conv_w1 ------------
            ps0 = ExitStack()
            pp = ps0.enter_context(tc.tile_pool(name="pp", bufs=4, space="PSUM"))
            psum_p = [pp.tile([128, HID], f32, tag="pp", name=f"psum_p{_i}")
                      for _i in range(RT)]
            for f in range(FT):
                nt_t = st3.tile([128, ROWS], fp16, tag="ath")
                nc.sync.dma_start(nt_t[:], nodet[f * 128:(f + 1) * 128, :])
                cw1_t = st3.tile([128, HID], fp16, tag="cw1")
                nc.sync.dma_start(cw1_t[:], cw1[f * 128:(f + 1) * 128, :])
                for i in range(RT):
                    nc.tensor.matmul(
                        psum_p[i][:],
                        lhsT=nt_t[:, i * 128:(i + 1) * 128],
                        rhs=cw1_t[:],
                        start=(f == 0), stop=(f == FT - 1),
                    )

            # ------- phase 3: merge candidates -> tau, top-32 -> deg, dinv ---
            deg_sb = const.tile([128, RT], f32, tag="deg")
            dinv_own = const.tile([128, RT], f32, tag="dinv_own")
            tmp1 = const.tile([128, RT], f32, tag="tmp1")
            tau_sb = const.tile([128, RT], f32, tag="tau")
            top32 = const.tile([128, RT, 32], f32, tag="top32")
            for i in range(RT):
                cand = btop[:, i, :, :]  # [128, NCH, 32] candidates
                z2 = wstage.tile([128, NCH * 32], f32, tag="z2")
                m8b = wstage.tile([128, 8], f32, tag="m8b")
                nc.vector.max(out=m8b[:], in_=cand)
                nc.vector.tensor_copy(top32[:, i, 0:8], m8b[:])
                nc.vector.match_replace(out=z2[:], in_to_replace=m8b[:],
                                        in_values=cand, imm_value=-1e30)
                for r in range(1, 4):
                    nc.vector.max(out=m8b[:], in_=z2[:])
                    nc.vector.tensor_copy(top32[:, i, 8 * r:8 * r + 8], m8b[:])
                    if r < 3:
                        nc.vector.match_replace(out=z2[:], in_to_replace=m8b[:],
                                                in_values=z2[:],
                                                imm_value=-1e30)
                nc.vector.tensor_copy(tau_sb[:, i:i + 1], m8b[:, 7:8])
                # deg = sum of the exact top-32 values; dinv guarded rsqrt
                nc.vector.reduce_sum(deg_sb[:, i:i + 1], top32[:, i, :],
                                     axis=mybir.AxisListType.X)
                nc.vector.tensor_scalar_max(tmp1[:, i:i + 1], deg_sb[:, i:i + 1],
                                            1e-12)
                nc.scalar.activation(tmp1[:, i:i + 1], tmp1[:, i:i + 1], AF.Sqrt)
                nc.vector.reciprocal(tmp1[:, i:i + 1], tmp1[:, i:i + 1])
                nc.vector.tensor_scalar(dinv_own[:, i:i + 1], deg_sb[:, i:i + 1],
                                        0.0, None, op0=ALU.is_gt)
                nc.vector.tensor_mul(dinv_own[:, i:i + 1], dinv_own[:, i:i + 1],
                                     tmp1[:, i:i + 1])

            # ------- phase 4: P rows scaled by dinv_own + deg -> AllGather ---
            p_sb = hold.tile([128, RT, PC], fp16, tag="p_sb")
            nc.vector.memset(p_sb[:, :, HID + 1:], 0.0)
            for i in range(RT):
                nc.scalar.activation(p_sb[:, i, 0:HID], psum_p[i][:], AF.Copy,
                                     scale=dinv_own[:, i:i + 1])
                nc.vector.tensor_copy(p_sb[:, i, HID:HID + 1],
                                      deg_sb[:, i:i + 1])
            ps0.close()
            nc.sync.dma_start(pdeg_shard.rearrange("(t p) c -> p t c", p=128),
                              p_sb[:])
            nc.gpsimd.collective_compute(
                "AllGather", ALU.bypass, replica_groups=GRP,
                ins=[pdeg_shard[:, :]], outs=[pdeg_full[:, :]],
            )

            # ------- phase 5: mask W in place (exact threshold, scale) -------
            for i in range(RT):
                for off, w in CHUNKS:
                    asl = adjs[:, i, off:off + w]
                    mk = stage.tile([128, 1024], f32, tag="scr")
                    nc.vector.tensor_scalar(mk[:, 0:w], asl,
                                            tau_sb[:, i:i + 1],
                                            dinv_own[:, i:i + 1],
                                            op0=ALU.is_ge, op1=ALU.mult)
                    nc.vector.tensor_mul(asl, asl, mk[:, 0:w])

            # pd/deg_all loads (gated on the AllGather by dram deps)
            pd = hold.tile([128, JT, HID], fp16, tag="pd")
            for half in range(2):
                jt0 = half * (JT // 2)
                nc.sync.dma_start(
                    pd[:, jt0:jt0 + JT // 2, :],
                    pdeg_full[half * (N // 2):(half + 1) * (N // 2), 0:HID]
                    .rearrange("(t p) c -> p t c", p=128))
            deg_a16 = const.tile([128, JT], fp16, tag="deg_a16")
            nc.sync.dma_start(deg_a16[:],
                              pdeg_full[:, HID:HID + 1]
                              .rearrange("(t p) c -> p (t c)", p=128))
            deg_all = const.tile([128, JT], f32, tag="deg_all")
            nc.vector.tensor_copy(deg_all[:], deg_a16[:])
            dinv_all = const.tile([128, JT], f32, tag="dinv_all")
            tmp2 = const.tile([128, JT], f32, tag="tmp2")
            nc.vector.tensor_scalar_max(tmp2[:], deg_all[:], 1e-12)
            nc.scalar.activation(tmp2[:], tmp2[:], AF.Sqrt)
            nc.vector.reciprocal(tmp2[:], tmp2[:])
            nc.vector.tensor_scalar(dinv_all[:], deg_all[:], 0.0, None,
                                    op0=ALU.is_gt)
            nc.vector.tensor_mul(dinv_all[:], dinv_all[:], tmp2[:])

            # ------- phase 6: W.T blocks (transpose) + msg1.T interleaved ----
            ps6 = ExitStack()
            pm = ps6.enter_context(tc.tile_pool(name="pm", bufs=2, space="PSUM"))
            ptst = ExitStack()
            pt = ptst.enter_context(tc.tile_pool(name="pt", bufs=4, space="PSUM"))
            wtd_sb = hold.tile([128, JT, ROWS], fp16, tag="wtd")
            obt = hold.tile([128, CT, ROWS], f32, tag="obt")
            psm = [pm.tile([128, ROWS], f32, tag="pm", name=f"psm{_c}")
                   for _c in range(CT)]
            for jt in range(JT):
                pst = pt.tile([128, ROWS], f32, tag="pt")
                for i in range(RT):
                    nc.tensor.transpose(pst[:, i * 128:(i + 1) * 128],
                                        adjs[:, i, jt * 128:(jt + 1) * 128],
                                        ident[:])
                nc.scalar.activation(wtd_sb[:, jt, :], pst[:], AF.Copy)
                for ct in range(CT):
                    nc.tensor.matmul(
                        psm[ct][:],
                        lhsT=pd[:, jt, ct * 128:(ct + 1) * 128],
                        rhs=wtd_sb[:, jt, :],
                        start=(jt == 0), stop=(jt == JT - 1),
                    )
            adjp.close()
            ptst.close()
            for ct in range(CT):
                nc.vector.tensor_scalar(obt[:, ct, :], psm[ct][:],
                                        b1c_sb[:, ct:ct + 1], None,
                                        op0=ALU.add)

            # ------- phase 7: BN stats (local) -> AllReduce; out1 AllGather --
            sq = hold.tile([128, CT, ROWS], f32, tag="sq")
            nc.vector.tensor_mul(sq[:], obt[:], obt[:])
            st_sb = const.tile([128, 4], f32, tag="st")
            for ct in range(CT):
                nc.vector.reduce_sum(st_sb[:, ct:ct + 1], obt[:, ct, :],
                                     axis=mybir.AxisListType.X)
                nc.vector.reduce_sum(st_sb[:, 2 + ct:3 + ct], sq[:, ct, :],
                                     axis=mybir.AxisListType.X)
            nc.sync.dma_start(stats_loc.rearrange("(t p) -> p t", p=128),
                              st_sb[:])
            nc.gpsimd.collective_compute(
                "AllReduce", ALU.add, replica_groups=GRP,
                ins=[stats_loc[:]], outs=[stats_red[:]],
            )
            o1s = hold.tile([128, CT, ROWS], fp16, tag="o1s")
            nc.vector.tensor_copy(o1s[:], obt[:])
            nc.sync.dma_start(o1_shard.rearrange("(ct p) i -> p ct i", p=128),
                              o1s[:])
            nc.gpsimd.collective_compute(
                "AllGather", ALU.bypass, replica_groups=GRP,
                ins=[o1_shard[:, :]], outs=[o1_full[:, :]],
            )

            # ------- phase 8: BN apply on gathered out1.T; Q = bn @ cw2 ------
            lstack = ExitStack()
            late = lstack.enter_context(tc.tile_pool(name="late", bufs=1))
            bnT = late.tile([128, CT, N], fp16, tag="bnT")
            o1v = o1_full.rearrange("(s q p) i -> q p s i", p=128, q=CT)
            for ct in range(CT):
                nc.sync.dma_start(bnT[:, ct, :], o1v[ct])
            str_sb = const.tile([128, 4], f32, tag="str")
            nc.sync.dma_start(str_sb[:],
                              stats_red.rearrange("(t p) -> p t", p=128))
            mean = const.tile([128, CT], f32, tag="mean")
            var = const.tile([128, CT], f32, tag="var")
            nc.vector.tensor_scalar_mul(mean[:], str_sb[:, 0:CT], 1.0 / N)
            nc.vector.tensor_scalar_mul(var[:], str_sb[:, CT:2 * CT], 1.0 / N)
            msq = const.tile([128, CT], f32, tag="msq")
            nc.vector.tensor_mul(msq[:], mean[:], mean[:])
            nc.vector.tensor_sub(var[:], var[:], msq[:])
            rstd = const.tile([128, CT], f32, tag="rstd")
            nc.vector.tensor_scalar_add(rstd[:], var[:], BN_EPS)
            nc.scalar.activation(rstd[:], rstd[:], AF.Sqrt)
            nc.vector.reciprocal(rstd[:], rstd[:])
            s_bn = const.tile([128, CT], f32, tag="s_bn")
            nc.vector.tensor_mul(s_bn[:], gam_sb[:], rstd[:])
            t_bn = const.tile([128, CT], f32, tag="t_bn")
            nc.vector.tensor_mul(t_bn[:], mean[:], s_bn[:])
            nc.vector.tensor_sub(t_bn[:], bet_sb[:], t_bn[:])
            for ct in range(CT):
                nc.scalar.activation(bnT[:, ct, :], bnT[:, ct, :], AF.Relu,
                                     bias=t_bn[:, ct:ct + 1],
                                     scale=s_bn[:, ct:ct + 1])

            qd = late.tile([128, JT, OUT], fp16, tag="qd")
            pq = ps6.enter_context(tc.tile_pool(name="pq", bufs=4, space="PSUM"))
            for jb in range(JT):
                psq = pq.tile([128, OUT], f32, tag="pq")
                for ct in range(CT):
                    nc.tensor.matmul(
                        psq[:],
                        lhsT=bnT[:, ct, jb * 128:(jb + 1) * 128],
                        rhs=cw2_sb[:, ct, :],
                        start=(ct == 0), stop=(ct == CT - 1),
                    )
                nc.scalar.activation(qd[:, jb, :], psq[:], AF.Copy,
                                     scale=dinv_all[:, jb:jb + 1])

            # ------- phase 9: out.T = msg2.T + b2c ---------------------------
            fsb = hold.tile([128, CT, ROWS], f32, tag="fsb")
            for ct in range(CT):
                psf = pm.tile([128, ROWS], f32, tag="pf")
                for jt in range(JT):
                    nc.tensor.matmul(
                        psf[:],
                        lhsT=qd[:, jt, ct * 128:(ct + 1) * 128],
                        rhs=wtd_sb[:, jt, :],
                        start=(jt == 0), stop=(jt == JT - 1),
                    )
                nc.vector.tensor_scalar(fsb[:, ct, :], psf[:],
                                        b2c_sb[:, ct:ct + 1], None,
                                        op0=ALU.add)
            nc.sync.dma_start(out.rearrange("(t p) i -> p t i", p=128), fsb[:])
            ps6.close()
            lstack.close()
            sstack.close()

    nc.compile()
    return nc


def _device_reset():
    """Tiny SPMD program to clear wedged device state after a crash."""
    nc = bacc.Bacc(None, target_bir_lowering=False)
    x = nc.declare_dram_parameter("x", [128, 128], dt.float32, isOutput=False)
    y = nc.declare_dram_parameter("y", [128, 128], dt.float32, isOutput=True)
    with tile.TileContext(nc) as tc:
        with tc.tile_pool(name="sb", bufs=1) as sb:
            t = sb.tile([128, 128], dt.float32, tag="t")
            nc.sync.dma_start(t[:], x[:, :])
            nc.vector.tensor_scalar_add(t[:], t[:], 1.0)
            nc.sync.dma_start(y[:, :], t[:])
    nc.compile()
    z = np.zeros((128, 128), np.float32)
    run_bass_kernel_spmd(nc, [{"x": z} for _ in range(N_CORES)],
                         list(range(N_CORES)))


def kernel(probs, bbox_coords, query_emb, node_emb,
           mlp_w1, mlp_b1, mlp_w2, mlp_b2,
           conv_w1, conv_b1, conv_w2, conv_b2,
           bn_gamma, bn_beta):
    global _CACHED_NC
    if _CACHED_NC is None:
        _CACHED_NC = _build()
    nc = _CACHED_NC

    f = np.float32

    def split_hl(x):
        hi = x.astype(np.float16)
        lo = (x - hi.astype(f)).astype(np.float16)
        return np.ascontiguousarray(hi), np.ascontiguousarray(lo)

    ew = np.concatenate([np.asarray(query_emb, f), np.asarray(probs, f),
                         np.asarray(bbox_coords, f)], axis=1)
    at_full = np.zeros((KIN_PAD, N), f)
    at_full[:KIN, :] = ew.T
    w1p = np.zeros((KIN_PAD, H_MLP), f)
    w1p[:KIN, :] = np.asarray(mlp_w1, f)
    node = np.asarray(node_emb, f)
    w1h_np, w1l_np = split_hl(w1p)
    w2h_np, w2l_np = split_hl(np.asarray(mlp_w2, f))

    shared = {
        "w1h": w1h_np, "w1l": w1l_np, "b1": np.asarray(mlp_b1, f),
        "w2h": w2h_np, "w2l": w2l_np,
        "b2": np.asarray(mlp_b2, f),
        "cw1": np.ascontiguousarray(np.asarray(conv_w1, f).astype(np.float16)),
        "b1c": np.asarray(conv_b1, f),
        "cw2": np.ascontiguousarray(np.asarray(conv_w2, f).astype(np.float16)),
        "b2c": np.asarray(conv_b2, f),
        "gamma": np.asarray(bn_gamma, f), "beta": np.asarray(bn_beta, f),
    }
    in_maps = []
    for c in range(N_CORES):
        sl = slice(c * ROWS, (c + 1) * ROWS)
        m = dict(shared)
        m["at_h"], m["at_l"] = split_hl(at_full[:, sl])
        m["nodet"] = np.ascontiguousarray(node[sl].T.astype(np.float16))
        in_maps.append(m)

    try:
        res = run_bass_kernel_spmd(nc, in_maps, list(range(N_CORES)),
                                   trace=TRACE)
    except Exception:
        # A freshly loaded NEFF occasionally leaves the device wedged
        # (NRT_EXEC_UNIT_UNRECOVERABLE). Running a trivial program clears
        # the state; retry once.
        try:
            _device_reset()
        except Exception:
            pass
        res = run_bass_kernel_spmd(nc, in_maps, list(range(N_CORES)),
                                   trace=TRACE)
    LAST_INFO["exec_time_ns"] = res.exec_time_ns
    LAST_INFO["mean_exec_time_ns"] = res.mean_exec_time_ns

    outp = np.empty((N, OUT), f)
    for c in range(N_CORES):
        outp[c * ROWS:(c + 1) * ROWS] = res.results[c]["out"].T
    return outp
